# revision 1
# baseline (speedup 1.0000x reference)
"""CrossDomainInterestLoss on 8 Trainium2 NeuronCores.

Strategy (hardcoded for bs=4096, dim=128):
  - Host: l2-normalize u/a/b (fp32), pre-transpose to [dim, rows] so the
    device needs no transposes, shard rows of u 4-way and cols (negatives)
    2-way -> 8 cores in a 4x2 grid.
  - Device (SPMD, identical program): for each 128-row chunk of u and each
    negatives matrix m in {A, B}:
      PE:  sim chunk [128, 2048] = uT_chunk.T @ mT  (4 matmuls into 4 PSUM banks)
      ACT: exp(sim/tau) with fused accum_out -> per-row partial pos sums
      DVE: relu(sim - margin) = (sim max m) add -m, fused accum -> relu sums
      DVE: is_gt(relu_tile, 0) on bf16 (4x mode), fused accum -> counts
    A few relu instructions run on ACT instead of DVE for engine balance.
  - Host: sum shard partials per row, analytically remove the diagonal
    contribution (device sums include j == i), then apply the exact
    reference formula on [4096] vectors.
"""

import numpy as np

import concourse.bass as bass
import concourse.mybir as mybir
from concourse import bacc, tile
from concourse import dve_ops as _dve_ops
from concourse.bass_utils import run_bass_kernel_spmd
from concourse.dve_ops import DveOp
from concourse.dve_spec import C0, C1, Spec, Src0, Zero, lower, relu, select
from concourse.dve_uop import DveOpSpec

TAU = 0.05
HARD_NEG_WEIGHT = 0.5
MARGIN = 0.3
BS = 4096
DIM = 128

R, C = 4, 2           # row-groups x col-groups = 8 cores
ROWS = BS // R        # u rows per core
COLS = BS // C        # negative rows per core (per matrix)
NRC = ROWS // 128     # 128-row chunks per core
NMM = COLS // 512     # matmuls per chunk per matrix

F32 = mybir.dt.float32
F32R = mybir.dt.float32r
BF16 = mybir.dt.bfloat16

# Custom DVE op: one pass over sim computing
#   out = relu(x - C0) + C1 * (x > C0),  accum = sum(out)
# With C1 = PACK_C the per-row accum packs both HNM quantities:
#   accum = relu_sum + PACK_C * count   (count <= ~30 per row here, so
#   PACK_C * count stays ~2^14 and fp32 accum keeps relu_sum precision).
PACK_C = 512.0


def _ref_relu_cnt_pack(in0, in1, s0, s1, imm2):
    r = np.maximum(in0.astype(np.float32) - s0, 0).astype(np.float32)
    g = ((in0 > s0).astype(np.float32) * s1).astype(np.float32)
    b = (r + g).astype(np.float32)
    return b, b.reshape(b.shape[0], -1).sum(axis=-1, keepdims=True).astype(np.float32)


def _get_packed_op():
    from operator import add as _add

    name = "RELU_CNT_PACK_ANT"
    for op in _dve_ops.OPS:
        if op.name == name:
            return op
    spec = Spec(
        body=relu(Src0 - C0) + select(Src0 > C0, C1, Zero),
        accum=_add,
        accum_init=Zero,
        reference=_ref_relu_cnt_pack,
    )
    row = _dve_ops._CUSTOM_DVE_ROW_BASE + len(_dve_ops.OPS)
    assert row < 0x20
    shas = {}
    for ver in ("v3", "v4"):
        try:
            uops = lower(spec, ver=ver)
            shas[ver] = DveOpSpec(
                name=name, opcode=row, uops=uops, rd1_en=False
            ).sha(ver)
        except Exception:
            pass
    op = DveOp(name, spec, subdim=False, uops_sha=shas)
    _dve_ops.OPS.append(op)
    _dve_ops._SUB_OPCODE_FOR_NAME[name] = row
    _dve_ops.CUSTOM_DVE_SPECS[name] = spec
    return op

# (rc, m) pairs whose relu+accum runs on ACT instead of DVE (engine balance)
ACT_RELU_PAIRS = set()

# PSUM group width: 1024 -> 2 banks x 4 bufs, 2048 -> 4 banks x 2 bufs.
GROUP_COLS = 1024
NG = COLS // GROUP_COLS  # accum columns per (rc, m)
NMM_G = GROUP_COLS // 512
PSUM_BUFS = 8192 // GROUP_COLS // 2

_BUILT = None
LAST_RESULTS = None  # BassKernelResults of the last run (for profiling)
TRACE = False
REPS = 1  # unrolled repetitions of the whole compute (wall-clock slope timing)
DYN_REPS = 0  # if > 0, wrap the compute in a For_i with this trip count


def _build_bass():
    global PACKED_OP
    PACKED_OP = _get_packed_op()
    nc = bacc.Bacc()

    # float32r: fp32 pre-rounded on the host to the PE's two-bf16 split so
    # matmuls stream at 1 cyc/col instead of fp32's 4.
    ut = nc.dram_tensor("ut", [DIM, ROWS], F32R, kind="ExternalInput")
    at = nc.dram_tensor("at", [DIM, COLS], F32R, kind="ExternalInput")
    bt = nc.dram_tensor("bt", [DIM, COLS], F32R, kind="ExternalInput")

    outs = {}
    for name in ("pos_a", "pos_b", "rsum_a", "rsum_b", "cnt_a", "cnt_b"):
        outs[name] = nc.dram_tensor(
            name, [128, NRC * NG], F32, kind="ExternalOutput"
        )

    with tile.TileContext(nc) as tc:
        with (
            tc.tile_pool(name="ops", bufs=1) as ops,
            tc.tile_pool(name="stats", bufs=1) as stats,
            tc.tile_pool(name="escr", bufs=2) as escr,
            tc.tile_pool(name="rscr", bufs=2) as rscr,
            tc.tile_pool(name="gscr", bufs=2) as gscr,
            tc.tile_pool(
                name="psum", bufs=PSUM_BUFS, space=bass.MemorySpace.PSUM
            ) as psum,
        ):
            ut_s = ops.tile([DIM, ROWS], F32R, tag="ut")
            at_s = ops.tile([DIM, COLS], F32R, tag="at")
            bt_s = ops.tile([DIM, COLS], F32R, tag="bt")
            # Loads split across the SP HWDGE path and the gpsimd SWDGE path
            # so ut and the first at half land in parallel and the first
            # matmul starts ~2.8us in.
            half = COLS // 2
            nc.gpsimd.dma_start(ut_s[:], ut[:])
            nc.sync.dma_start(at_s[:, :512], at[:, :512])
            nc.sync.dma_start(at_s[:, 512:half], at[:, 512:half])
            nc.sync.dma_start(at_s[:, half:], at[:, half:])
            nc.sync.dma_start(bt_s[:, :half], bt[:, :half])
            nc.sync.dma_start(bt_s[:, half:], bt[:, half:])

            st = {
                n: stats.tile([128, NRC * NG], F32, tag=n, name=n) for n in outs
            }
            # Dummy 1-element exp as the first ACT instruction: the compiler
            # inserts LoadActFuncSet right before it, so the ~1.3us table
            # load overlaps the input DMAs instead of the first real exp.
            warm = stats.tile([128, 1], F32, tag="warm", name="warm")
            nc.scalar.activation(
                warm[:],
                nc.const_aps.tensor(0.0, (128, 1), F32),
                mybir.ActivationFunctionType.Exp,
            )
            neg_margin = stats.tile([128, 1], F32, tag="neg_margin")
            nc.gpsimd.memset(neg_margin[:], -MARGIN)
            # Zero stats so columns never written on device (cnt under the
            # packed op; odd columns in RING_MODE) read as 0.
            for n in outs:
                nc.gpsimd.memset(st[n][:], 0.0)
            neg = {0: at_s, 1: bt_s}
            sfx = {0: "a", 1: "b"}

            def emit_mm(lhsT, m, g):
                sim = psum.tile([128, GROUP_COLS], F32, tag="sim", name="sim")
                for n in range(NMM_G):
                    j0 = g * GROUP_COLS + n * 512
                    nc.tensor.matmul(
                        sim[:, n * 512 : (n + 1) * 512],
                        lhsT,
                        neg[m][:, j0 : j0 + 512],
                        start=True,
                        stop=True,
                    )
                return sim

            def emit_exp(rc, m, sim, g):
                # exp(sim/tau), fused fp32 row-sum -> pos partials; the bf16
                # out tile itself is unused.
                col = slice(rc * NG + g, rc * NG + g + 1)
                e_t = escr.tile([128, GROUP_COLS], BF16, tag="e", name="e")
                nc.scalar.activation(
                    e_t[:],
                    sim[:],
                    mybir.ActivationFunctionType.Exp,
                    scale=1.0 / TAU,
                    accum_out=st["pos_" + sfx[m]][:, col],
                )

            def emit_hnm(rc, m, sim, g):
                # One DVE pass packs relu_sum + PACK_C*count into the accum
                # (host unpacks). ACT path (engine balance) computes true
                # relu sums + a cheap 4x DVE count on the bf16 out.
                col = slice(rc * NG + g, rc * NG + g + 1)
                r_t = rscr.tile([128, GROUP_COLS], BF16, tag="r", name="r")
                if (rc, m) in ACT_RELU_PAIRS:
                    nc.scalar.activation(
                        r_t[:],
                        sim[:],
                        mybir.ActivationFunctionType.Relu,
                        bias=neg_margin[:],
                        accum_out=st["rsum_" + sfx[m]][:, col],
                    )
                    g_t = gscr.tile([128, GROUP_COLS], BF16, tag="g", name="g")
                    nc.vector.tensor_scalar(
                        g_t[:],
                        r_t[:],
                        0.0,
                        None,
                        mybir.AluOpType.is_gt,
                        mybir.AluOpType.add,
                        accum_out=st["cnt_" + sfx[m]][:, col],
                    )
                else:
                    nc.vector._custom_dve(
                        PACKED_OP,
                        out=r_t[:],
                        in0=sim[:],
                        s0=MARGIN,
                        s1=PACK_C,
                        accum_out=st["rsum_" + sfx[m]][:, col],
                    )

            def body():
                for rc in range(NRC):
                    lhsT = ut_s[:, rc * 128 : (rc + 1) * 128]
                    for m in (0, 1):
                        for g in range(NG):
                            sim = emit_mm(lhsT, m, g)
                            emit_exp(rc, m, sim, g)
                            emit_hnm(rc, m, sim, g)

            if DYN_REPS > 0:
                with tc.For_i(0, DYN_REPS, 1):
                    body()
            else:
                for _rep in range(REPS):
                    body()

            for name, dram in outs.items():
                nc.sync.dma_start(dram[:], st[name][:])

    nc.compile()
    return nc


def _get_built():
    global _BUILT
    if _BUILT is None:
        _BUILT = _build_bass()
    return _BUILT


def gather_partials(results):
    """Combine per-core outputs into per-row [BS] vectors and unpack the
    packed relu/count accumulators."""

    def gather(name):
        out = np.zeros(BS, dtype=np.float64)
        for k in range(8):
            rg = k // C
            arr = results[k][name].astype(np.float64)  # [128, NRC*NG]
            blk = arr.T.reshape(NRC, NG, 128).sum(axis=1).reshape(ROWS)
            out[rg * ROWS : (rg + 1) * ROWS] += blk
        return out

    pos_A, pos_B = gather("pos_a"), gather("pos_b")
    rsum_A, rsum_B = gather("rsum_a"), gather("rsum_b")
    cnt_A, cnt_B = gather("cnt_a"), gather("cnt_b")

    # Unpack relu_sum + PACK_C*count for chunks handled by the packed DVE op.
    rcs = np.arange(BS) % ROWS // 128
    for rsum, cnt, m in ((rsum_A, cnt_A, 0), (rsum_B, cnt_B, 1)):
        for rc in range(NRC):
            if (rc, m) in ACT_RELU_PAIRS:
                continue
            rows = rcs == rc
            packed = rsum[rows]
            c = np.floor(packed / PACK_C + 0.25)
            rsum[rows] = packed - PACK_C * c
            cnt[rows] = c
    return pos_A, pos_B, rsum_A, rsum_B, cnt_A, cnt_B


def _l2norm(x):
    n = np.linalg.norm(x.astype(np.float64), axis=1, keepdims=True)
    return (x.astype(np.float64) / np.maximum(n, 1e-12)).astype(np.float32)


def _round_f32r(x):
    """Round fp32 to the PE's float32r representation: the exactly-split
    sum of two bf16s (hi + lo)."""
    import ml_dtypes

    hi = x.astype(ml_dtypes.bfloat16).astype(np.float32)
    lo = (x - hi).astype(ml_dtypes.bfloat16).astype(np.float32)
    return hi + lo


def kernel(user_interest, reg_A_emb, reg_B_emb):
    global LAST_RESULTS
    u = _l2norm(np.asarray(user_interest, dtype=np.float32))
    a = _l2norm(np.asarray(reg_A_emb, dtype=np.float32))
    b = _l2norm(np.asarray(reg_B_emb, dtype=np.float32))

    u = _round_f32r(u)
    a = _round_f32r(a)
    b = _round_f32r(b)
    in_maps = []
    for k in range(8):
        rg, cg = k // C, k % C
        in_maps.append(
            {
                "ut": np.ascontiguousarray(u[rg * ROWS : (rg + 1) * ROWS].T),
                "at": np.ascontiguousarray(a[cg * COLS : (cg + 1) * COLS].T),
                "bt": np.ascontiguousarray(b[cg * COLS : (cg + 1) * COLS].T),
            }
        )

    nc = _get_built()
    res = run_bass_kernel_spmd(nc, in_maps, list(range(8)), trace=TRACE)
    LAST_RESULTS = res

    pos_A, pos_B, rsum_A, rsum_B, cnt_A, cnt_B = gather_partials(res.results)


    # Remove the diagonal contribution from the HNM sums (device included it).
    u64, a64, b64 = u.astype(np.float64), a.astype(np.float64), b.astype(np.float64)
    d_A = np.sum(u64 * a64, axis=1)
    d_B = np.sum(u64 * b64, axis=1)
    rsum_A -= np.maximum(d_A - MARGIN, 0.0)
    rsum_B -= np.maximum(d_B - MARGIN, 0.0)
    cnt_A -= (d_A > MARGIN).astype(np.float64)
    cnt_B -= (d_B > MARGIN).astype(np.float64)

    denom = np.maximum(pos_A + pos_B, 1e-9)
    loss_A = -np.mean(np.log(pos_A / denom))
    loss_B = -np.mean(np.log(pos_B / denom))
    base_loss = (loss_A + loss_B) / 2.0

    def hnm(rsum, cnt):
        has = cnt > 0.5
        n_rows = np.count_nonzero(has)
        if n_rows == 0:
            return 0.0
        total = np.sum(rsum[has] + MARGIN * cnt[has])
        return total / n_rows

    weighted_hard = 0.5 * hnm(rsum_A, cnt_A) + 1.0 * hnm(rsum_B, cnt_B)
    total = base_loss + (
        HARD_NEG_WEIGHT * weighted_hard if abs(weighted_hard) > 1e-9 else 0.0
    )
    return np.float32(total)



# revision 12
# speedup vs baseline: 3.2509x; 3.2509x over previous
"""CrossDomainInterestLoss on 8 Trainium2 NeuronCores.

Strategy (hardcoded for bs=4096, dim=128), v2 "mask-matmul" design:
  - Host: l2-normalize u/a/b (fp32), pre-transpose to [dim, rows], shard
    rows of u 4-way and cols (negatives) 2-way -> 8 cores in a 4x2 grid.
    Also ship a bf16 copy of u chunks (row-major) for the mask matmuls.
  - Device (SPMD): flat pipeline over k = (m, h, i): matrix m in {A,B},
    col-half h in {0,1} (1024 negatives), row-chunk i in 0..7:
      PE : sim tile [128,1024] = ut_i.T @ neg[m][:, h]   (f32r, 1 matmul)
      ACT: e = bf16(exp(sim/tau)) -> SBUF (no accum; 1x rate is ACT's floor)
      DVE: mask m = (e > exp(margin/tau)) bf16, accum -> cnt   (4x mode)
      DVE: possum pass over e, accum -> pos                    (4x mode)
      PE : MT[m,h] += u16_i.T @ m   (accumulate over i in PSUM [128d,1024j])
      DVE: after i=7, tensor_tensor_reduce(at_slice * MT) -> frob col
           (sum_j (u^T M)[d,j] * aT[d,j] = sum over hard negatives of sim)
  - Host: pos sums -> base loss exactly as reference; HNM from
    total_m = frob_m - diag terms, nrows from exact per-row counts.

  Per-core steady-state engine busy (TimelineSim): ACT ~33us (bound),
  DVE ~25us, PE ~28us. All PSUM: 2 sim bufs + 2 MT bufs = 8 banks.
"""

import numpy as np

import concourse.bass as bass
import concourse.mybir as mybir
from concourse import bacc, tile
from concourse.bass_utils import run_bass_kernel_spmd

TAU = 0.05
HARD_NEG_WEIGHT = 0.5
MARGIN = 0.3
BS = 4096
DIM = 128

R, C = 4, 2           # row-groups x col-groups = 8 cores
ROWS = BS // R        # u rows per core (1024)
COLS = BS // C        # negative rows per core per matrix (2048)
NRC = ROWS // 128     # 128-row chunks per core (8)
NH = COLS // 1024     # col halves per matrix (2)
FD = 1024             # tile free dim

E_MARGIN = float(np.exp(MARGIN / TAU))  # mask threshold on e = exp(sim/tau)

F32 = mybir.dt.float32
F32R = mybir.dt.float32r
BF16 = mybir.dt.bfloat16

_BUILT = None
LAST_RESULTS = None
TRACE = False
REPS = 1
DYN_REPS = 0
# debug bisect flags
EN_MT = True    # mask matmuls + TTR
EN_TTR = True   # tensor_tensor_reduce
EN_MASK = True  # DVE mask pass
EN_POS = True   # DVE possum pass


def _build_bass():
    nc = bacc.Bacc()

    ut = nc.dram_tensor("ut", [DIM, ROWS], F32R, kind="ExternalInput")
    u16 = nc.dram_tensor("u16", [128, ROWS], BF16, kind="ExternalInput")
    at = nc.dram_tensor("at", [DIM, COLS], F32R, kind="ExternalInput")
    bt = nc.dram_tensor("bt", [DIM, COLS], F32R, kind="ExternalInput")

    NPOS = 2 * NH * NRC  # 32 accum columns for pos/cnt
    pos_o = nc.dram_tensor("pos", [128, NPOS], F32, kind="ExternalOutput")
    cnt_o = nc.dram_tensor("cnt", [128, NPOS], F32, kind="ExternalOutput")
    frob_o = nc.dram_tensor("frob", [128, 2 * NH], F32, kind="ExternalOutput")

    with tile.TileContext(nc) as tc:
        with (
            tc.tile_pool(name="ops", bufs=1) as ops,
            tc.tile_pool(name="stats", bufs=1) as stats,
            tc.tile_pool(name="escr", bufs=3) as escr,
            tc.tile_pool(name="mscr", bufs=3) as mscr,
            tc.tile_pool(name="pscr", bufs=2) as pscr,
            tc.tile_pool(name="fscr", bufs=2) as fscr,
            tc.tile_pool(
                name="psum", bufs=2, space=bass.MemorySpace.PSUM
            ) as psum,
            tc.tile_pool(
                name="psum_mt", bufs=2, space=bass.MemorySpace.PSUM
            ) as psum_mt,
        ):
            ut_s = ops.tile([DIM, ROWS], F32R, tag="ut")
            u16_s = ops.tile([128, ROWS], BF16, tag="u16")
            at_s = ops.tile([DIM, COLS], F32R, tag="at")
            bt_s = ops.tile([DIM, COLS], F32R, tag="bt")
            # Split loads across SWDGE (gpsimd) and HWDGE (sync) queues so
            # the first matmul's operands land early.
            nc.gpsimd.dma_start(ut_s[:], ut[:])
            nc.gpsimd.dma_start(u16_s[:], u16[:])
            nc.sync.dma_start(at_s[:, :1024], at[:, :1024])
            nc.sync.dma_start(at_s[:, 1024:], at[:, 1024:])
            nc.sync.dma_start(bt_s[:, :1024], bt[:, :1024])
            nc.sync.dma_start(bt_s[:, 1024:], bt[:, 1024:])

            pos_t = stats.tile([128, NPOS], F32, tag="pos", name="pos")
            cnt_t = stats.tile([128, NPOS], F32, tag="cnt", name="cnt")
            frob_t = stats.tile([128, 2 * NH], F32, tag="frob", name="frob")
            # Dummy 1-element exp: pulls the ~1.3us LoadActFuncSet into the
            # DMA prologue instead of before the first real exp.
            warm = stats.tile([128, 1], F32, tag="warm", name="warm")
            nc.scalar.activation(
                warm[:],
                nc.const_aps.tensor(0.0, (128, 1), F32),
                mybir.ActivationFunctionType.Exp,
            )
            for t in (pos_t, cnt_t, frob_t):
                nc.gpsimd.memset(t[:], 0.0)

            neg = {0: at_s, 1: bt_s}

            def body():
                # flat pipeline over (m, h, i); MT matmul for step k-1 is
                # emitted after the sim matmul of step k so PE never waits
                # on DVE's mask of the current step.
                steps = [
                    (m, h, i)
                    for m in (0, 1)
                    for h in range(NH)
                    for i in range(NRC)
                ]
                masks = {}
                mt_ps = {}

                def emit_mt(k):
                    if not (EN_MT and EN_MASK):
                        return
                    m, h, i = steps[k]
                    if i == 0:
                        mt_ps[(m, h)] = psum_mt.tile(
                            [128, FD], F32, tag="mt", name="mt"
                        )
                    mk = masks.pop(k)
                    for n in range(FD // 512):
                        nc.tensor.matmul(
                            mt_ps[(m, h)][:, n * 512 : (n + 1) * 512],
                            u16_s[:, i * 128 : (i + 1) * 128],
                            mk[:, n * 512 : (n + 1) * 512],
                            start=(i == 0),
                            stop=(i == NRC - 1),
                        )
                    if i == NRC - 1 and EN_TTR:
                        f_t = fscr.tile([128, FD], F32, tag="f", name="f")
                        col = 2 * m + h
                        nc.vector.scalar_tensor_tensor(
                            f_t[:],
                            neg[m][:, h * FD : (h + 1) * FD].bitcast(F32),
                            1.0,
                            mt_ps.pop((m, h))[:],
                            mybir.AluOpType.mult,
                            mybir.AluOpType.mult,
                            accum_out=frob_t[:, col : col + 1],
                        )

                for k, (m, h, i) in enumerate(steps):
                    sim = psum.tile([128, FD], F32, tag="sim", name="sim")
                    for n in range(FD // 512):
                        nc.tensor.matmul(
                            sim[:, n * 512 : (n + 1) * 512],
                            ut_s[:, i * 128 : (i + 1) * 128],
                            neg[m][:, h * FD + n * 512 : h * FD + (n + 1) * 512],
                            start=True,
                            stop=True,
                        )
                    if k > 1:
                        emit_mt(k - 2)

                    e_t = escr.tile([128, FD], BF16, tag="e", name="e")
                    nc.scalar.activation(
                        e_t[:],
                        sim[:],
                        mybir.ActivationFunctionType.Exp,
                        scale=1.0 / TAU,
                    )

                    col = (2 * m + h) * NRC + i
                    if EN_MASK:
                        m_t = mscr.tile([128, FD], BF16, tag="m", name="m")
                        nc.vector.tensor_scalar(
                            m_t[:],
                            e_t[:],
                            E_MARGIN,
                            None,
                            mybir.AluOpType.is_gt,
                            mybir.AluOpType.add,
                            accum_out=cnt_t[:, col : col + 1],
                        )
                        masks[k] = m_t
                    if EN_POS:
                        p_t = pscr.tile([128, FD], BF16, tag="p", name="p")
                        nc.vector.tensor_scalar(
                            p_t[:],
                            e_t[:],
                            1.0,
                            None,
                            mybir.AluOpType.mult,
                            mybir.AluOpType.add,
                            accum_out=pos_t[:, col : col + 1],
                        )
                emit_mt(len(steps) - 2)
                emit_mt(len(steps) - 1)

            if DYN_REPS > 0:
                with tc.For_i(0, DYN_REPS, 1):
                    body()
            else:
                for _rep in range(REPS):
                    body()

            nc.sync.dma_start(pos_o[:], pos_t[:])
            nc.sync.dma_start(cnt_o[:], cnt_t[:])
            nc.sync.dma_start(frob_o[:], frob_t[:])

    nc.compile()
    return nc


def _get_built():
    global _BUILT
    if _BUILT is None:
        _BUILT = _build_bass()
    return _BUILT


def _l2norm(x):
    n = np.linalg.norm(x.astype(np.float64), axis=1, keepdims=True)
    return (x.astype(np.float64) / np.maximum(n, 1e-12)).astype(np.float32)


def _round_f32r(x):
    """Round fp32 to the PE's float32r representation (hi + lo bf16 split)."""
    import ml_dtypes

    hi = x.astype(ml_dtypes.bfloat16).astype(np.float32)
    lo = (x - hi).astype(ml_dtypes.bfloat16).astype(np.float32)
    return hi + lo


def gather_partials(results):
    """Combine per-core outputs into full-batch vectors/scalars."""
    pos = {0: np.zeros(BS), 1: np.zeros(BS)}
    cnt = {0: np.zeros(BS), 1: np.zeros(BS)}
    frob = {0: 0.0, 1: 0.0}
    for k in range(8):
        rg = k // C
        po = results[k]["pos"].astype(np.float64)  # [128, 32]
        co = results[k]["cnt"].astype(np.float64)
        fo = results[k]["frob"].astype(np.float64)  # [128, 4]
        for m in (0, 1):
            for h in range(NH):
                for i in range(NRC):
                    col = (2 * m + h) * NRC + i
                    rows = slice(rg * ROWS + i * 128, rg * ROWS + (i + 1) * 128)
                    pos[m][rows] += po[:, col]
                    cnt[m][rows] += co[:, col]
                frob[m] += fo[:, 2 * m + h].sum()
    return pos, cnt, frob


def kernel(user_interest, reg_A_emb, reg_B_emb):
    global LAST_RESULTS
    import ml_dtypes

    u = _l2norm(np.asarray(user_interest, dtype=np.float32))
    a = _l2norm(np.asarray(reg_A_emb, dtype=np.float32))
    b = _l2norm(np.asarray(reg_B_emb, dtype=np.float32))

    uf = _round_f32r(u)
    af = _round_f32r(a)
    bf = _round_f32r(b)
    u16 = u.astype(ml_dtypes.bfloat16)

    in_maps = []
    for k in range(8):
        rg, cg = k // C, k % C
        urows = slice(rg * ROWS, (rg + 1) * ROWS)
        # u16 layout: [:, i*128 + d] = u[rg*ROWS + i*128 + p, d]
        u16_k = np.ascontiguousarray(
            u16[urows].reshape(NRC, 128, DIM).transpose(1, 0, 2).reshape(128, ROWS)
        )
        in_maps.append(
            {
                "ut": np.ascontiguousarray(uf[urows].T),
                "u16": u16_k,
                "at": np.ascontiguousarray(af[cg * COLS : (cg + 1) * COLS].T),
                "bt": np.ascontiguousarray(bf[cg * COLS : (cg + 1) * COLS].T),
            }
        )

    nc = _get_built()
    res = run_bass_kernel_spmd(nc, in_maps, list(range(8)), trace=TRACE)
    LAST_RESULTS = res

    pos, cnt, frob = gather_partials(res.results)
    pos_A, pos_B = pos[0], pos[1]
    cnt_A, cnt_B = cnt[0], cnt[1]

    # Remove diagonal contributions (device masks include j == i).
    u64, a64, b64 = u.astype(np.float64), a.astype(np.float64), b.astype(np.float64)
    d_A = np.sum(u64 * a64, axis=1)
    d_B = np.sum(u64 * b64, axis=1)
    hard_dA = d_A > MARGIN
    hard_dB = d_B > MARGIN
    cnt_A = cnt_A - hard_dA.astype(np.float64)
    cnt_B = cnt_B - hard_dB.astype(np.float64)
    total_A = frob[0] - d_A[hard_dA].sum()
    total_B = frob[1] - d_B[hard_dB].sum()

    denom = np.maximum(pos_A + pos_B, 1e-9)
    loss_A = -np.mean(np.log(pos_A / denom))
    loss_B = -np.mean(np.log(pos_B / denom))
    base_loss = (loss_A + loss_B) / 2.0

    def hnm(total, cnt):
        n_rows = np.count_nonzero(cnt > 0.5)
        if n_rows == 0:
            return 0.0
        return total / n_rows

    weighted_hard = 0.5 * hnm(total_A, cnt_A) + 1.0 * hnm(total_B, cnt_B)
    total = base_loss + (
        HARD_NEG_WEIGHT * weighted_hard if abs(weighted_hard) > 1e-9 else 0.0
    )
    return np.float32(total)
